# revision 6
# baseline (speedup 1.0000x reference)
"""GPS layer (GCN + per-graph MHA + FFN, BatchNorm eval) on 8 trn2 cores.

Sharding: 16 graphs data-parallel, 2 graphs per core (block-diagonal
adjacency => no cross-core edges). Each core runs an identical Bass/Tile
program on its slice.

Host prep is layout only (slicing, transposes, bf16 casts) plus
densifying the per-graph adjacency into A^T (the on-device scatter
primitives — gpsimd local_scatter / indirect DMA with batched offsets —
are not supported by this walrus toolchain; densification places
edge values, summing the ~0.2% duplicate (row,col) pairs).

Device layout: activations feature-major [d, nodes] so BatchNorm and
biases are per-partition ACT affines; SpMM is dense PE matmuls against
A^T; attention computes transposed scores S^T=[k',q] per head, exp on
ACT without max subtraction (|scores|/sqrt(dh) < 1 for this data
regime), softmax denominator via a ones-column in the v operand, and
1/Z is broadcast across partitions with a K=1 PE matmul.
"""

import numpy as np
import ml_dtypes

BF16 = ml_dtypes.bfloat16

B, N, D, H = 16, 512, 256, 8
EP = 16384
NCORES = 8
GPC = B // NCORES            # graphs per core = 2
NODES = N * GPC              # nodes per core = 1024
DH = D // H                  # 32
BN_EPS = 1e-5
INV_SQRT_DH = float(1.0 / np.sqrt(DH))
NB = NODES // 128            # node blocks per core = 8
NGB = N // 128               # node blocks per graph = 4
DB = D // 128                # feature blocks = 2

_prog_cache = {}


def _split_waits(nc, mybir, max_waits=1):
    """walrus CoreV3 rejects >1 sync wait per instruction; move excess
    waits onto preceding NOPs."""
    for bb in nc.main_func.blocks:
        new_instrs = []
        for ins in bb.instructions:
            si = ins.sync_info
            waits = list(si.on_wait) if si is not None and si.on_wait else []
            if len(waits) > max_waits:
                keep = waits[-max_waits:]
                for i, w in enumerate(waits[:-max_waits]):
                    new_instrs.append(
                        mybir.InstNoOp(
                            name=f"{ins.name}-ws{i}",
                            sync_info=mybir.SyncInfo(on_wait=[w], on_update=[]),
                            bass_nofuse=True,
                            engine=ins.engine,
                        )
                    )
                ins.sync_info = mybir.SyncInfo(
                    on_wait=keep, on_update=list(si.on_update or [])
                )
            new_instrs.append(ins)
        bb.instructions[:] = new_instrs


def _build_program():
    import concourse.bass as bass
    import concourse.tile as tile
    import concourse.mybir as mybir

    f32 = mybir.dt.float32
    bf = mybir.dt.bfloat16
    AF = mybir.ActivationFunctionType

    nc = bass.Bass()
    dp = nc.declare_dram_parameter
    xT_f = dp("xT_f", [D, NODES], f32, isOutput=False)
    xT_b = dp("xT_b", [D, NODES], bf, isOutput=False)
    wgcnT = dp("wgcnT", [D, D], bf, isOutput=False)
    ipwT = dp("ipwT", [D, 3 * D], bf, isOutput=False)
    ipb = dp("ipb", [3 * D], f32, isOutput=False)
    ipbv = dp("ipbv", [DH, H], f32, isOutput=False)
    opw2 = dp("opw2", [DH, H * D], bf, isOutput=False)
    opb = dp("opb", [D], f32, isOutput=False)
    w1T = dp("w1T", [D, 4 * D], bf, isOutput=False)
    b1 = dp("b1", [4 * D], f32, isOutput=False)
    w2T = dp("w2T", [4 * D, D], bf, isOutput=False)
    b2 = dp("b2", [D], f32, isOutput=False)
    bnp = dp("bnp", [12, D], f32, isOutput=False)  # bn{1,2,3} x (g,b,m,v)
    at_in = dp("AT", [NODES, N], bf, isOutput=False)
    outp = dp("out", [D, NODES], f32, isOutput=True)

    with tile.TileContext(nc) as tc:
        with (
            tc.tile_pool(name="const", bufs=1) as cp,
            tc.tile_pool(name="act", bufs=1) as ap_,
            tc.tile_pool(name="work", bufs=3) as wp,
            tc.tile_pool(name="psum", bufs=2, space="PSUM") as pp,
            tc.tile_pool(name="psum_s", bufs=2, space="PSUM") as pps,
            tc.tile_pool(name="psum_c", bufs=2, space="PSUM") as ppc,
        ):
            # ---------- constant loads ----------
            t_xTf = cp.tile([128, DB, NODES], f32, tag="xTf")
            nc.sync.dma_start(t_xTf[:], xT_f.rearrange("(a p) n -> p a n", p=128))
            t_xTb = cp.tile([128, DB, NODES], bf, tag="xTb")
            nc.sync.dma_start(t_xTb[:], xT_b.rearrange("(a p) n -> p a n", p=128))
            t_wgcn = cp.tile([128, DB, D], bf, tag="wgcn")
            nc.sync.dma_start(t_wgcn[:], wgcnT.rearrange("(a p) e -> p a e", p=128))
            t_ipw = cp.tile([128, DB, 3 * D], bf, tag="ipw")
            nc.sync.dma_start(t_ipw[:], ipwT.rearrange("(a p) e -> p a e", p=128))
            t_opw2 = cp.tile([DH, H * D], bf, tag="opw2")
            nc.sync.dma_start(t_opw2[:], opw2[:])
            t_w1 = cp.tile([128, DB, 4 * D], bf, tag="w1")
            nc.sync.dma_start(t_w1[:], w1T.rearrange("(a p) e -> p a e", p=128))
            t_w2 = cp.tile([128, 8, D], bf, tag="w2")
            nc.sync.dma_start(t_w2[:], w2T.rearrange("(a p) e -> p a e", p=128))
            t_ipb = cp.tile([128, 6], f32, tag="ipb")
            nc.sync.dma_start(t_ipb[:], ipb.rearrange("(a p) -> p a", p=128))
            t_ipbv = cp.tile([DH, H], f32, tag="ipbv")
            nc.sync.dma_start(t_ipbv[:], ipbv[:])
            t_opb = cp.tile([128, 2], f32, tag="opb")
            nc.sync.dma_start(t_opb[:], opb.rearrange("(a p) -> p a", p=128))
            t_b1 = cp.tile([128, 8], f32, tag="b1")
            nc.sync.dma_start(t_b1[:], b1.rearrange("(a p) -> p a", p=128))
            t_b2 = cp.tile([128, 2], f32, tag="b2")
            nc.sync.dma_start(t_b2[:], b2.rearrange("(a p) -> p a", p=128))
            t_bnp = cp.tile([128, 12, DB], f32, tag="bnp")
            nc.sync.dma_start(t_bnp[:], bnp.rearrange("r (a p) -> p r a", p=128))
            t_AT = cp.tile([128, NB, N], bf, tag="AT")
            nc.sync.dma_start(t_AT[:], at_in.rearrange("(cb p) r -> p cb r", p=128))
            # ones row at partition 32 for the 1/Z cross-partition broadcast
            t_onz = cp.tile([DH + 1, DH], f32, tag="onz")
            nc.vector.memset(t_onz[:], 1.0)

            # ---------- BN scale/shift: s = g/sqrt(v+eps), t = b - m*s ----
            g_ap = t_bnp[:, 0::4, :]
            b_ap = t_bnp[:, 1::4, :]
            m_ap = t_bnp[:, 2::4, :]
            v_ap = t_bnp[:, 3::4, :]
            t_ve = ap_.tile([128, 3, DB], f32, tag="veps")
            nc.vector.tensor_scalar_add(t_ve[:], v_ap, BN_EPS)
            t_std = ap_.tile([128, 3, DB], f32, tag="std")
            nc.scalar.activation(t_std[:], t_ve[:], AF.Sqrt)
            t_rstd = ap_.tile([128, 3, DB], f32, tag="rstd")
            nc.vector.reciprocal(t_rstd[:], t_std[:])
            t_s = ap_.tile([128, 3, DB], f32, tag="bns")
            nc.vector.tensor_mul(t_s[:], g_ap, t_rstd[:])
            t_ms = ap_.tile([128, 3, DB], f32, tag="bnms")
            nc.vector.tensor_mul(t_ms[:], m_ap, t_s[:])
            t_t = ap_.tile([128, 3, DB], f32, tag="bnt")
            nc.vector.tensor_sub(t_t[:], b_ap, t_ms[:])

            # ---------- hl = x @ w_gcn.T  (node-major [c, d], bf16) -------
            t_hl = ap_.tile([128, NB, D], bf, tag="hl")
            for cb in range(NB):
                ps = pp.tile([128, D], f32, space="PSUM", tag="ps")
                for kd in range(DB):
                    nc.tensor.matmul(
                        ps[:],
                        t_xTb[:, kd, cb * 128 : (cb + 1) * 128],
                        t_wgcn[:, kd, :],
                        start=(kd == 0),
                        stop=(kd == DB - 1),
                    )
                nc.scalar.activation(t_hl[:, cb, :], ps[:], AF.Copy)

            # ---------- agg^T = (A @ hl)^T ; gelu; +x; BN1 ----------
            t_x1f = ap_.tile([128, DB, NODES], f32, tag="x1f")
            t_x1b = ap_.tile([128, DB, NODES], bf, tag="x1b")
            for g in range(GPC):
                for db in range(DB):
                    ps = pp.tile([128, N], f32, space="PSUM", tag="ps")
                    for kc in range(NGB):
                        cb = g * NGB + kc
                        nc.tensor.matmul(
                            ps[:],
                            t_hl[:, cb, db * 128 : (db + 1) * 128],
                            t_AT[:, cb, :],
                            start=(kc == 0),
                            stop=(kc == NGB - 1),
                        )
                    ns = slice(g * N, (g + 1) * N)
                    t_gl = wp.tile([128, N], f32, tag="gelu1")
                    nc.scalar.activation(t_gl[:], ps[:], AF.Gelu)
                    t_x1 = wp.tile([128, N], f32, tag="x1tmp")
                    nc.vector.tensor_add(t_x1[:], t_gl[:], t_xTf[:, db, ns])
                    nc.scalar.activation(
                        t_x1f[:, db, ns], t_x1[:], AF.Identity,
                        bias=t_t[:, 0, db:db+1], scale=t_s[:, 0, db:db+1],
                    )
                    nc.scalar.activation(
                        t_x1b[:, db, ns], t_x1[:], AF.Identity,
                        bias=t_t[:, 0, db:db+1], scale=t_s[:, 0, db:db+1],
                    )

            # ---------- attention (per graph) ----------
            t_x2f = ap_.tile([128, DB, NODES], f32, tag="x2f")
            t_x2b = ap_.tile([128, DB, NODES], bf, tag="x2b")
            for g in range(GPC):
                ns = slice(g * N, (g + 1) * N)
                # q^T,k^T feature-major: [128, 4(eb), N]
                t_qk = wp.tile([128, 4, N], bf, tag="qk")
                for eb in range(4):
                    ps = pp.tile([128, N], f32, space="PSUM", tag="ps")
                    for kd in range(DB):
                        nc.tensor.matmul(
                            ps[:],
                            t_ipw[:, kd, eb * 128 : (eb + 1) * 128],
                            t_x1b[:, kd, ns],
                            start=(kd == 0),
                            stop=(kd == DB - 1),
                        )
                    nc.scalar.activation(
                        t_qk[:, eb, :], ps[:], AF.Identity, bias=t_ipb[:, eb:eb+1]
                    )
                # v node-major + ones column: [128, NGB(nb), H, DH+1]
                t_va = wp.tile([128, NGB, H, DH + 1], bf, tag="vaug")
                nc.vector.memset(t_va[:, :, :, DH : DH + 1], 1.0)
                for nb in range(NGB):
                    ps = pp.tile([128, D], f32, space="PSUM", tag="ps")
                    nlo = g * N + nb * 128
                    for kd in range(DB):
                        nc.tensor.matmul(
                            ps[:],
                            t_x1b[:, kd, nlo : nlo + 128],
                            t_ipw[:, kd, 2 * D : 3 * D],
                            start=(kd == 0),
                            stop=(kd == DB - 1),
                        )
                    nc.scalar.activation(
                        t_va[:, nb, :, 0:DH],
                        ps[:].rearrange("p (h d) -> p h d", h=H),
                        AF.Copy,
                    )
                # per-head: S^T -> exp -> ctx(+Z row) -> 1/Z -> ctx_h
                t_ctxh = wp.tile([DH, H, N], bf, tag="ctxh")
                for h in range(H):
                    hb = 2 + h // 4          # k e-block in t_qk
                    po = 32 * (h % 4)        # partition offset of head rows
                    t_es = wp.tile([128, NGB, N], bf, tag="expS")
                    for kb in range(NGB):
                        ps = pps.tile([128, N], f32, space="PSUM", tag="ps_s")
                        nc.tensor.matmul(
                            ps[:],
                            t_qk[po : po + 32, hb, kb * 128 : (kb + 1) * 128],
                            t_qk[po : po + 32, hb - 2, :],
                            start=True,
                            stop=True,
                            tile_position=(po, 0),
                        )
                        nc.scalar.activation(
                            t_es[:, kb, :], ps[:], AF.Exp, scale=INV_SQRT_DH
                        )
                    psc = ppc.tile([DH + 1, N], f32, space="PSUM", tag="ps_c")
                    for kb in range(NGB):
                        nc.tensor.matmul(
                            psc[:],
                            t_va[:, kb, h, :],
                            t_es[:, kb, :],
                            start=(kb == 0),
                            stop=(kb == NGB - 1),
                        )
                    # 1/Z at partition DH, broadcast to partitions 0..DH-1
                    t_zr = wp.tile([DH + 1, N], f32, tag="zr")
                    nc.vector.reciprocal(
                        t_zr[DH : DH + 1, :], psc[DH : DH + 1, :]
                    )
                    ps_zb = ppc.tile([DH, N], f32, space="PSUM", tag="ps_zb")
                    nc.tensor.matmul(
                        ps_zb[:],
                        t_onz[DH : DH + 1, :],
                        t_zr[DH : DH + 1, :],
                        start=True,
                        stop=True,
                        tile_position=(DH, 0),
                    )
                    t_zbc = wp.tile([DH, N], f32, tag="zbc")
                    nc.scalar.activation(t_zbc[:], ps_zb[:], AF.Copy)
                    t_cn = wp.tile([DH, N], f32, tag="ctxn")
                    nc.vector.tensor_mul(t_cn[:], psc[0:DH, :], t_zbc[:])
                    nc.scalar.activation(
                        t_ctxh[:, h, :], t_cn[:], AF.Identity,
                        bias=t_ipbv[:, h:h+1],
                    )
                # out_proj (accumulate heads, K=32) + residual + BN2
                for db in range(DB):
                    ps = pp.tile([128, N], f32, space="PSUM", tag="ps")
                    for h in range(H):
                        nc.tensor.matmul(
                            ps[:],
                            t_opw2[:, h * D + db * 128 : h * D + (db + 1) * 128],
                            t_ctxh[:, h, :],
                            start=(h == 0),
                            stop=(h == H - 1),
                            tile_position=(0, 0),
                        )
                    t_ha = wp.tile([128, N], f32, tag="hattn")
                    nc.scalar.activation(
                        t_ha[:], ps[:], AF.Identity, bias=t_opb[:, db:db+1]
                    )
                    t_x2 = wp.tile([128, N], f32, tag="x2tmp")
                    nc.vector.tensor_add(t_x2[:], t_ha[:], t_x1f[:, db, ns])
                    nc.scalar.activation(
                        t_x2f[:, db, ns], t_x2[:], AF.Identity,
                        bias=t_t[:, 1, db:db+1], scale=t_s[:, 1, db:db+1],
                    )
                    nc.scalar.activation(
                        t_x2b[:, db, ns], t_x2[:], AF.Identity,
                        bias=t_t[:, 1, db:db+1], scale=t_s[:, 1, db:db+1],
                    )

            # ---------- FFN ----------
            t_h1 = ap_.tile([128, 8, NODES], bf, tag="h1")
            for mb in range(8):
                for g in range(GPC):
                    ns = slice(g * N, (g + 1) * N)
                    ps = pp.tile([128, N], f32, space="PSUM", tag="ps")
                    for kd in range(DB):
                        nc.tensor.matmul(
                            ps[:],
                            t_w1[:, kd, mb * 128 : (mb + 1) * 128],
                            t_x2b[:, kd, ns],
                            start=(kd == 0),
                            stop=(kd == DB - 1),
                        )
                    nc.scalar.activation(
                        t_h1[:, mb, ns], ps[:], AF.Gelu, bias=t_b1[:, mb:mb+1]
                    )
            t_out = ap_.tile([128, DB, NODES], f32, tag="outT")
            for g in range(GPC):
                ns = slice(g * N, (g + 1) * N)
                for db in range(DB):
                    ps = pp.tile([128, N], f32, space="PSUM", tag="ps")
                    for kb in range(8):
                        nc.tensor.matmul(
                            ps[:],
                            t_w2[:, kb, db * 128 : (db + 1) * 128],
                            t_h1[:, kb, ns],
                            start=(kb == 0),
                            stop=(kb == 7),
                        )
                    t_h2 = wp.tile([128, N], f32, tag="h2tmp")
                    nc.scalar.activation(
                        t_h2[:], ps[:], AF.Identity, bias=t_b2[:, db:db+1]
                    )
                    t_x3 = wp.tile([128, N], f32, tag="x3tmp")
                    nc.vector.tensor_add(t_x3[:], t_h2[:], t_x2f[:, db, ns])
                    nc.scalar.activation(
                        t_out[:, db, ns], t_x3[:], AF.Identity,
                        bias=t_t[:, 2, db:db+1], scale=t_s[:, 2, db:db+1],
                    )
            nc.sync.dma_start(outp.rearrange("(a p) n -> p a n", p=128), t_out[:])

    _split_waits(nc, mybir, 1)
    return nc


def kernel(**inputs):
    from concourse.bass_utils import run_bass_kernel_spmd

    x = np.asarray(inputs["x"], np.float32)
    er = np.asarray(inputs["edge_rows"]).astype(np.int64)
    ec = np.asarray(inputs["edge_cols"]).astype(np.int64)
    ev = np.asarray(inputs["edge_vals"], np.float32)

    ipw = np.asarray(inputs["in_proj_w"], np.float32)
    ipb = np.asarray(inputs["in_proj_b"], np.float32)
    opw = np.asarray(inputs["out_proj_w"], np.float32)
    bnp = np.stack(
        [
            np.asarray(inputs[f"bn{k}_{f}"], np.float32)
            for k in (1, 2, 3)
            for f in ("g", "b", "m", "v")
        ]
    )

    # out_proj_w^T regrouped per head at partitions 0..DH-1:
    # opw2[dh, h*D + e] = opw[e, h*DH + dh]
    opw2 = (
        np.ascontiguousarray(opw.T.reshape(H, DH, D).transpose(1, 0, 2))
        .reshape(DH, H * D)
        .astype(BF16)
    )

    shared = {
        "wgcnT": np.asarray(inputs["w_gcn"], np.float32).T.astype(BF16).copy(),
        "ipwT": ipw.T.astype(BF16).copy(),
        "ipb": ipb,
        "ipbv": np.ascontiguousarray(ipb[2 * D :].reshape(H, DH).T),
        "opw2": opw2,
        "opb": np.asarray(inputs["out_proj_b"], np.float32),
        "w1T": np.asarray(inputs["w1"], np.float32).T.astype(BF16).copy(),
        "b1": np.asarray(inputs["b1"], np.float32),
        "w2T": np.asarray(inputs["w2"], np.float32).T.astype(BF16).copy(),
        "b2": np.asarray(inputs["b2"], np.float32),
        "bnp": bnp,
    }

    in_maps = []
    for c in range(NCORES):
        base = c * NODES
        elo, ehi = GPC * c * EP, GPC * (c + 1) * EP
        r = (er[elo:ehi] - base).astype(np.int64)
        cc = (ec[elo:ehi] - base).astype(np.int64)
        v = ev[elo:ehi]
        # dense A^T: AT[c, r%N] = sum of vals of edges (r, c); block-diag
        at = np.zeros((NODES, N), np.float32)
        np.add.at(at, (cc, r % N), v)
        xT = np.ascontiguousarray(x[base : base + NODES].T)
        in_maps.append(
            {
                "xT_f": xT.astype(np.float32),
                "xT_b": xT.astype(BF16),
                "AT": at.astype(BF16),
                **shared,
            }
        )

    if "prog" not in _prog_cache:
        _prog_cache["prog"] = _build_program()
    nc = _prog_cache["prog"]
    _prog_cache["last_in_maps"] = in_maps

    res = run_bass_kernel_spmd(nc, in_maps, list(range(NCORES)))
    out = np.empty((B * N, D), np.float32)
    for c in range(NCORES):
        out[c * NODES : (c + 1) * NODES] = res.results[c]["out"].T
    return out


# revision 8
# speedup vs baseline: 485.1804x; 485.1804x over previous
"""GPS layer (GCN + per-graph MHA + FFN, BatchNorm eval) on 8 trn2 cores.

Sharding: 16 graphs data-parallel, 2 graphs per core (block-diagonal
adjacency => no cross-core edges). Each core runs an identical Bass/Tile
program on its slice.

Host prep is layout only (slicing, transposes, bf16 casts) plus
densifying the per-graph adjacency into A^T (the on-device scatter
primitives — gpsimd local_scatter / indirect DMA with batched offsets —
are not supported by this walrus toolchain; densification places
edge values, summing the ~0.2% duplicate (row,col) pairs).

Device layout: activations feature-major [d, nodes] so BatchNorm and
biases are per-partition ACT affines; SpMM is dense PE matmuls against
A^T; attention computes transposed scores S^T=[k',q] per head, exp on
ACT without max subtraction (|scores|/sqrt(dh) < 1 for this data
regime), softmax denominator via a ones-column in the v operand, and
1/Z is broadcast across partitions with a K=1 PE matmul.
"""

import numpy as np
import ml_dtypes

BF16 = ml_dtypes.bfloat16

B, N, D, H = 16, 512, 256, 8
EP = 16384
NCORES = 8
GPC = B // NCORES            # graphs per core = 2
NODES = N * GPC              # nodes per core = 1024
DH = D // H                  # 32
BN_EPS = 1e-5
INV_SQRT_DH = float(1.0 / np.sqrt(DH))
NB = NODES // 128            # node blocks per core = 8
NGB = N // 128               # node blocks per graph = 4
DB = D // 128                # feature blocks = 2

_prog_cache = {}


def _split_waits(nc, mybir, max_waits=1):
    """walrus CoreV3 rejects >1 sync wait per instruction; move excess
    waits onto preceding NOPs."""
    for bb in nc.main_func.blocks:
        new_instrs = []
        for ins in bb.instructions:
            si = ins.sync_info
            waits = list(si.on_wait) if si is not None and si.on_wait else []
            if len(waits) > max_waits:
                keep = waits[-max_waits:]
                for i, w in enumerate(waits[:-max_waits]):
                    new_instrs.append(
                        mybir.InstNoOp(
                            name=f"{ins.name}-ws{i}",
                            sync_info=mybir.SyncInfo(on_wait=[w], on_update=[]),
                            bass_nofuse=True,
                            engine=ins.engine,
                        )
                    )
                ins.sync_info = mybir.SyncInfo(
                    on_wait=keep, on_update=list(si.on_update or [])
                )
            new_instrs.append(ins)
        bb.instructions[:] = new_instrs


def _build_program():
    import concourse.bass as bass
    import concourse.tile as tile
    import concourse.mybir as mybir

    f32 = mybir.dt.float32
    bf = mybir.dt.bfloat16
    AF = mybir.ActivationFunctionType

    nc = bass.Bass()
    dp = nc.declare_dram_parameter
    xT_f = dp("xT_f", [D, NODES], f32, isOutput=False)
    xT_b = dp("xT_b", [D, NODES], bf, isOutput=False)
    wgcnT = dp("wgcnT", [D, D], bf, isOutput=False)
    ipwT = dp("ipwT", [D, 3 * D], bf, isOutput=False)
    ipb = dp("ipb", [3 * D], f32, isOutput=False)
    ipbv = dp("ipbv", [DH, H], f32, isOutput=False)
    opw2 = dp("opw2", [DH, H * D], bf, isOutput=False)
    opb = dp("opb", [D], f32, isOutput=False)
    w1T = dp("w1T", [D, 4 * D], bf, isOutput=False)
    b1 = dp("b1", [4 * D], f32, isOutput=False)
    w2T = dp("w2T", [4 * D, D], bf, isOutput=False)
    b2 = dp("b2", [D], f32, isOutput=False)
    bnp = dp("bnp", [12, D], f32, isOutput=False)  # bn{1,2,3} x (g,b,m,v)
    at_in = dp("AT", [NODES, N], bf, isOutput=False)
    outp = dp("out", [D, NODES], f32, isOutput=True)

    with tile.TileContext(nc) as tc:
        with (
            tc.tile_pool(name="const", bufs=1) as cp,
            tc.tile_pool(name="act", bufs=1) as ap_,
            tc.tile_pool(name="work", bufs=2) as wp,
            tc.tile_pool(name="psum", bufs=2, space="PSUM") as pp,
            tc.tile_pool(name="psum_s", bufs=4, space="PSUM") as pps,
            tc.tile_pool(name="psum_c", bufs=2, space="PSUM") as ppc,
        ):
            # ---------- constant loads ----------
            t_xTf = cp.tile([128, DB, NODES], f32, tag="xTf")
            nc.sync.dma_start(t_xTf[:], xT_f.rearrange("(a p) n -> p a n", p=128))
            t_xTb = cp.tile([128, DB, NODES], bf, tag="xTb")
            nc.sync.dma_start(t_xTb[:], xT_b.rearrange("(a p) n -> p a n", p=128))
            t_wgcn = cp.tile([128, DB, D], bf, tag="wgcn")
            nc.sync.dma_start(t_wgcn[:], wgcnT.rearrange("(a p) e -> p a e", p=128))
            t_ipw = cp.tile([128, DB, 3 * D], bf, tag="ipw")
            nc.sync.dma_start(t_ipw[:], ipwT.rearrange("(a p) e -> p a e", p=128))
            t_opw2 = cp.tile([DH, H * D], bf, tag="opw2")
            nc.sync.dma_start(t_opw2[:], opw2[:])
            t_w1 = cp.tile([128, DB, 4 * D], bf, tag="w1")
            nc.sync.dma_start(t_w1[:], w1T.rearrange("(a p) e -> p a e", p=128))
            t_w2 = cp.tile([128, 8, D], bf, tag="w2")
            nc.sync.dma_start(t_w2[:], w2T.rearrange("(a p) e -> p a e", p=128))
            t_ipb = cp.tile([128, 6], f32, tag="ipb")
            nc.sync.dma_start(t_ipb[:], ipb.rearrange("(a p) -> p a", p=128))
            t_ipbv = cp.tile([DH, H], f32, tag="ipbv")
            nc.sync.dma_start(t_ipbv[:], ipbv[:])
            t_opb = cp.tile([128, 2], f32, tag="opb")
            nc.sync.dma_start(t_opb[:], opb.rearrange("(a p) -> p a", p=128))
            t_b1 = cp.tile([128, 8], f32, tag="b1")
            nc.sync.dma_start(t_b1[:], b1.rearrange("(a p) -> p a", p=128))
            t_b2 = cp.tile([128, 2], f32, tag="b2")
            nc.sync.dma_start(t_b2[:], b2.rearrange("(a p) -> p a", p=128))
            t_bnp = cp.tile([128, 12, DB], f32, tag="bnp")
            nc.sync.dma_start(t_bnp[:], bnp.rearrange("r (a p) -> p r a", p=128))
            t_AT = cp.tile([128, NB, N], bf, tag="AT")
            nc.sync.dma_start(t_AT[:], at_in.rearrange("(cb p) r -> p cb r", p=128))
            # ones row at partition 32 for the 1/Z cross-partition broadcast
            t_onz = cp.tile([DH + 1, DH], f32, tag="onz")
            nc.vector.memset(t_onz[:], 1.0)

            # ---------- BN scale/shift: s = g/sqrt(v+eps), t = b - m*s ----
            g_ap = t_bnp[:, 0::4, :]
            b_ap = t_bnp[:, 1::4, :]
            m_ap = t_bnp[:, 2::4, :]
            v_ap = t_bnp[:, 3::4, :]
            t_ve = ap_.tile([128, 3, DB], f32, tag="veps")
            nc.vector.tensor_scalar_add(t_ve[:], v_ap, BN_EPS)
            t_std = ap_.tile([128, 3, DB], f32, tag="std")
            nc.scalar.activation(t_std[:], t_ve[:], AF.Sqrt)
            t_rstd = ap_.tile([128, 3, DB], f32, tag="rstd")
            nc.vector.reciprocal(t_rstd[:], t_std[:])
            t_s = ap_.tile([128, 3, DB], f32, tag="bns")
            nc.vector.tensor_mul(t_s[:], g_ap, t_rstd[:])
            t_ms = ap_.tile([128, 3, DB], f32, tag="bnms")
            nc.vector.tensor_mul(t_ms[:], m_ap, t_s[:])
            t_t = ap_.tile([128, 3, DB], f32, tag="bnt")
            nc.vector.tensor_sub(t_t[:], b_ap, t_ms[:])

            # ---------- hl = x @ w_gcn.T  (node-major [c, d], bf16) -------
            t_hl = ap_.tile([128, NB, D], bf, tag="hl")
            for cb in range(NB):
                ps = pp.tile([128, D], f32, space="PSUM", tag="ps")
                for kd in range(DB):
                    nc.tensor.matmul(
                        ps[:],
                        t_xTb[:, kd, cb * 128 : (cb + 1) * 128],
                        t_wgcn[:, kd, :],
                        start=(kd == 0),
                        stop=(kd == DB - 1),
                    )
                nc.scalar.activation(t_hl[:, cb, :], ps[:], AF.Copy)

            # ---------- agg^T = (A @ hl)^T ; gelu; +x; BN1 ----------
            t_x1f = ap_.tile([128, DB, NODES], f32, tag="x1f")
            t_x1b = ap_.tile([128, DB, NODES], bf, tag="x1b")
            for g in range(GPC):
                for db in range(DB):
                    ps = pp.tile([128, N], f32, space="PSUM", tag="ps")
                    for kc in range(NGB):
                        cb = g * NGB + kc
                        nc.tensor.matmul(
                            ps[:],
                            t_hl[:, cb, db * 128 : (db + 1) * 128],
                            t_AT[:, cb, :],
                            start=(kc == 0),
                            stop=(kc == NGB - 1),
                        )
                    ns = slice(g * N, (g + 1) * N)
                    t_gl = wp.tile([128, N], f32, tag="gelu1")
                    nc.scalar.activation(t_gl[:], ps[:], AF.Gelu)
                    t_x1 = wp.tile([128, N], f32, tag="x1tmp")
                    nc.vector.tensor_add(t_x1[:], t_gl[:], t_xTf[:, db, ns])
                    nc.scalar.activation(
                        t_x1f[:, db, ns], t_x1[:], AF.Identity,
                        bias=t_t[:, 0, db:db+1], scale=t_s[:, 0, db:db+1],
                    )
                    nc.vector.tensor_copy(t_x1b[:, db, ns], t_x1f[:, db, ns])

            # ---------- attention (per graph) ----------
            t_x2f = ap_.tile([128, DB, NODES], f32, tag="x2f")
            t_x2b = ap_.tile([128, DB, NODES], bf, tag="x2b")
            for g in range(GPC):
                ns = slice(g * N, (g + 1) * N)
                # q^T,k^T feature-major: [128, 4(eb), N]
                t_qk = wp.tile([128, 4, N], bf, tag="qk")
                for eb in range(4):
                    ps = pp.tile([128, N], f32, space="PSUM", tag="ps")
                    for kd in range(DB):
                        nc.tensor.matmul(
                            ps[:],
                            t_ipw[:, kd, eb * 128 : (eb + 1) * 128],
                            t_x1b[:, kd, ns],
                            start=(kd == 0),
                            stop=(kd == DB - 1),
                        )
                    nc.scalar.activation(
                        t_qk[:, eb, :], ps[:], AF.Identity, bias=t_ipb[:, eb:eb+1]
                    )
                # v node-major + ones column: [128, NGB(nb), H, DH+1]
                t_va = wp.tile([128, NGB, H, DH + 1], bf, tag="vaug")
                nc.vector.memset(t_va[:, :, :, DH : DH + 1], 1.0)
                for nb in range(NGB):
                    ps = pp.tile([128, D], f32, space="PSUM", tag="ps")
                    nlo = g * N + nb * 128
                    for kd in range(DB):
                        nc.tensor.matmul(
                            ps[:],
                            t_x1b[:, kd, nlo : nlo + 128],
                            t_ipw[:, kd, 2 * D : 3 * D],
                            start=(kd == 0),
                            stop=(kd == DB - 1),
                        )
                    nc.scalar.activation(
                        t_va[:, nb, :, 0:DH],
                        ps[:].rearrange("p (h d) -> p h d", h=H),
                        AF.Copy,
                    )
                # scores+exp for all heads, kb-major: consecutive matmuls
                # hit different PE row-groups (tile_position) and overlap
                t_ctxh = wp.tile([DH, H, N], bf, tag="ctxh")
                t_esA = ap_.tile([128, H, NGB, N], bf, tag="esA")
                for kb in range(NGB):
                    for h in range(H):
                        hb = 2 + h // 4
                        po = 32 * (h % 4)
                        ps = pps.tile([128, N], f32, space="PSUM", tag="ps_s")
                        nc.tensor.matmul(
                            ps[:],
                            t_qk[po : po + 32, hb, kb * 128 : (kb + 1) * 128],
                            t_qk[po : po + 32, hb - 2, :],
                            start=True,
                            stop=True,
                            tile_position=(po, 0),
                        )
                        nc.scalar.activation(
                            t_esA[:, h, kb, :], ps[:], AF.Exp, scale=INV_SQRT_DH
                        )
                for h in range(H):
                    psc = ppc.tile([DH + 1, N], f32, space="PSUM", tag="ps_c")
                    for kb in range(NGB):
                        nc.tensor.matmul(
                            psc[:],
                            t_va[:, kb, h, :],
                            t_esA[:, h, kb, :],
                            start=(kb == 0),
                            stop=(kb == NGB - 1),
                        )
                    # 1/Z at partition DH, broadcast to partitions 0..DH-1
                    t_zr = wp.tile([DH + 1, N], f32, tag="zr")
                    nc.vector.reciprocal(
                        t_zr[DH : DH + 1, :], psc[DH : DH + 1, :]
                    )
                    ps_zb = ppc.tile([DH, N], f32, space="PSUM", tag="ps_c")
                    nc.tensor.matmul(
                        ps_zb[:],
                        t_onz[DH : DH + 1, :],
                        t_zr[DH : DH + 1, :],
                        start=True,
                        stop=True,
                        tile_position=(DH, 0),
                    )
                    t_zbc = wp.tile([DH, N], f32, tag="zbc")
                    nc.vector.tensor_copy(t_zbc[:], ps_zb[:])
                    t_cn = wp.tile([DH, N], f32, tag="ctxn")
                    nc.vector.tensor_mul(t_cn[:], psc[0:DH, :], t_zbc[:])
                    nc.scalar.activation(
                        t_ctxh[:, h, :], t_cn[:], AF.Identity,
                        bias=t_ipbv[:, h:h+1],
                    )
                # out_proj (accumulate heads, K=32) + residual + BN2
                for db in range(DB):
                    ps = pp.tile([128, N], f32, space="PSUM", tag="ps")
                    for h in range(H):
                        nc.tensor.matmul(
                            ps[:],
                            t_opw2[:, h * D + db * 128 : h * D + (db + 1) * 128],
                            t_ctxh[:, h, :],
                            start=(h == 0),
                            stop=(h == H - 1),
                            tile_position=(0, 0),
                        )
                    t_ha = wp.tile([128, N], f32, tag="hattn")
                    nc.scalar.activation(
                        t_ha[:], ps[:], AF.Identity, bias=t_opb[:, db:db+1]
                    )
                    t_x2 = wp.tile([128, N], f32, tag="x2tmp")
                    nc.vector.tensor_add(t_x2[:], t_ha[:], t_x1f[:, db, ns])
                    nc.scalar.activation(
                        t_x2f[:, db, ns], t_x2[:], AF.Identity,
                        bias=t_t[:, 1, db:db+1], scale=t_s[:, 1, db:db+1],
                    )
                    nc.vector.tensor_copy(t_x2b[:, db, ns], t_x2f[:, db, ns])

            # ---------- FFN ----------
            t_h1 = ap_.tile([128, 8, NODES], bf, tag="h1")
            for mb in range(8):
                for g in range(GPC):
                    ns = slice(g * N, (g + 1) * N)
                    ps = pp.tile([128, N], f32, space="PSUM", tag="ps")
                    for kd in range(DB):
                        nc.tensor.matmul(
                            ps[:],
                            t_w1[:, kd, mb * 128 : (mb + 1) * 128],
                            t_x2b[:, kd, ns],
                            start=(kd == 0),
                            stop=(kd == DB - 1),
                        )
                    nc.scalar.activation(
                        t_h1[:, mb, ns], ps[:], AF.Gelu, bias=t_b1[:, mb:mb+1]
                    )
            t_out = ap_.tile([128, DB, NODES], f32, tag="outT")
            for g in range(GPC):
                ns = slice(g * N, (g + 1) * N)
                for db in range(DB):
                    ps = pp.tile([128, N], f32, space="PSUM", tag="ps")
                    for kb in range(8):
                        nc.tensor.matmul(
                            ps[:],
                            t_w2[:, kb, db * 128 : (db + 1) * 128],
                            t_h1[:, kb, ns],
                            start=(kb == 0),
                            stop=(kb == 7),
                        )
                    t_h2 = wp.tile([128, N], f32, tag="h2tmp")
                    nc.scalar.activation(
                        t_h2[:], ps[:], AF.Identity, bias=t_b2[:, db:db+1]
                    )
                    t_x3 = wp.tile([128, N], f32, tag="x3tmp")
                    nc.vector.tensor_add(t_x3[:], t_h2[:], t_x2f[:, db, ns])
                    nc.scalar.activation(
                        t_out[:, db, ns], t_x3[:], AF.Identity,
                        bias=t_t[:, 2, db:db+1], scale=t_s[:, 2, db:db+1],
                    )
            nc.sync.dma_start(outp.rearrange("(a p) n -> p a n", p=128), t_out[:])

    _split_waits(nc, mybir, 1)
    return nc


def kernel(**inputs):
    from concourse.bass_utils import run_bass_kernel_spmd

    x = np.asarray(inputs["x"], np.float32)
    er = np.asarray(inputs["edge_rows"]).astype(np.int64)
    ec = np.asarray(inputs["edge_cols"]).astype(np.int64)
    ev = np.asarray(inputs["edge_vals"], np.float32)

    ipw = np.asarray(inputs["in_proj_w"], np.float32)
    ipb = np.asarray(inputs["in_proj_b"], np.float32)
    opw = np.asarray(inputs["out_proj_w"], np.float32)
    bnp = np.stack(
        [
            np.asarray(inputs[f"bn{k}_{f}"], np.float32)
            for k in (1, 2, 3)
            for f in ("g", "b", "m", "v")
        ]
    )

    # out_proj_w^T regrouped per head at partitions 0..DH-1:
    # opw2[dh, h*D + e] = opw[e, h*DH + dh]
    opw2 = (
        np.ascontiguousarray(opw.T.reshape(H, DH, D).transpose(1, 0, 2))
        .reshape(DH, H * D)
        .astype(BF16)
    )

    shared = {
        "wgcnT": np.asarray(inputs["w_gcn"], np.float32).T.astype(BF16).copy(),
        "ipwT": ipw.T.astype(BF16).copy(),
        "ipb": ipb,
        "ipbv": np.ascontiguousarray(ipb[2 * D :].reshape(H, DH).T),
        "opw2": opw2,
        "opb": np.asarray(inputs["out_proj_b"], np.float32),
        "w1T": np.asarray(inputs["w1"], np.float32).T.astype(BF16).copy(),
        "b1": np.asarray(inputs["b1"], np.float32),
        "w2T": np.asarray(inputs["w2"], np.float32).T.astype(BF16).copy(),
        "b2": np.asarray(inputs["b2"], np.float32),
        "bnp": bnp,
    }

    in_maps = []
    for c in range(NCORES):
        base = c * NODES
        elo, ehi = GPC * c * EP, GPC * (c + 1) * EP
        r = (er[elo:ehi] - base).astype(np.int64)
        cc = (ec[elo:ehi] - base).astype(np.int64)
        v = ev[elo:ehi]
        # dense A^T: AT[c, r%N] = sum of vals of edges (r, c); block-diag
        at = np.zeros((NODES, N), np.float32)
        np.add.at(at, (cc, r % N), v)
        xT = np.ascontiguousarray(x[base : base + NODES].T)
        in_maps.append(
            {
                "xT_f": xT.astype(np.float32),
                "xT_b": xT.astype(BF16),
                "AT": at.astype(BF16),
                **shared,
            }
        )

    if "prog" not in _prog_cache:
        _prog_cache["prog"] = _build_program()
    nc = _prog_cache["prog"]
    _prog_cache["last_in_maps"] = in_maps

    res = run_bass_kernel_spmd(nc, in_maps, list(range(NCORES)))
    out = np.empty((B * N, D), np.float32)
    for c in range(NCORES):
        out[c * NODES : (c + 1) * NODES] = res.results[c]["out"].T
    return out
